# revision 1
# baseline (speedup 1.0000x reference)
"""CapsuleLayer dynamic-routing kernel for 8 TRN2 NeuronCores.

Problem: inputs [256,1152,8] f32, W [1152,10,8,16] f32, bias [1,1152,10,1] f32.
  u_hat = einsum('bid,icdv->bicv', inputs, W)
  3 rounds of routing (softmax over c, weighted sum over i, squash over v).
Output: [256, 10, 16] f32.

Sharding: 2-way batch x 4-way input-capsule (i) grid over 8 cores.
Core k: batch half k//4 (128 rows), i-quarter k%4 (288 i's).
Per-round partial sums over i are combined with an AllReduce over each
group of 4 cores ([0..3] and [4..7]). Output halves read from cores 0, 4.

Per-core: partitions = batch (128). u_hat kept in SBUF as bf16
[128, 288*160] in (i, c, v) free order. u_hat generated by PE matmuls:
4 i's per matmul via K=32 block-diagonal weights (base-partition must be
0/32/64, so (i,d) rows are packed in 96-row tiles), evicted from PSUM by
DVE/ACT copies. Routing passes run chunk-wise on DVE (bf16 2x mode) with
some chunks offloaded to GPSIMD; reductions are halving-add trees (2x)
rather than 1x tensor_reduce.
"""

import sys

if "/opt/trn_rl_repo" not in sys.path:
    sys.path.insert(0, "/opt/trn_rl_repo")

import numpy as np
import ml_dtypes

import concourse.bass as bass
from concourse import bacc, mybir, tile
from concourse.bass_utils import run_bass_kernel_spmd

F32 = mybir.dt.float32
BF16 = mybir.dt.bfloat16
AX = mybir.AxisListType
ALU = mybir.AluOpType
ACTF = mybir.ActivationFunctionType

B, I, D, C, V = 256, 1152, 8, 10, 16
CV = C * V                     # 160
NB = 128                       # batch rows per core
IQ = 288                       # i's per core
NG = IQ // 4                   # 72 groups of 4 i's (K=32 block-diag matmuls)
NT = NG // 3                   # 24 tiles of 96 partition-rows
EPS = 1e-7

RCH = 16                       # i's per routing chunk
NRC = IQ // RCH                # 18
RC = RCH * CV                  # 2560 elems per routing chunk

REPLICA_GROUPS = [[0, 1, 2, 3], [4, 5, 6, 7]]


def _ap(ap, dims):
    """Build an AP with explicit [step, count] free dims (partition dim kept)."""
    return bass.AP(ap.tensor, ap.offset, [list(ap.ap[0])] + [list(d) for d in dims])


def _squash(nc, pool, s_in, v_out):
    """v = (|s|^2/(1+|s|^2)) * s / sqrt(|s|^2 + EPS), norms over v (16).

    s_in: [128, 160] f32 SBUF AP. Writes v_out (bf16 for routing rounds,
    f32 for the final output round)."""
    sq = pool.tile([128, CV], F32, tag="sq")
    n2 = pool.tile([128, C], F32, tag="n2")
    n2e = pool.tile([128, C], F32, tag="n2e")
    qs = pool.tile([128, C], F32, tag="qs")
    mm = pool.tile([128, C], F32, tag="mm")
    rm = pool.tile([128, C], F32, tag="rm")
    fc = pool.tile([128, C], F32, tag="fc")
    nc.vector.tensor_mul(sq[:], s_in, s_in)
    nc.vector.tensor_reduce(
        n2[:], sq[:].rearrange("p (c v) -> p c v", v=V), axis=AX.X, op=ALU.add
    )
    # f = n2 / ((1+n2) * sqrt(n2+eps))
    nc.vector.tensor_scalar_add(n2e[:], n2[:], EPS)
    nc.scalar.activation(qs[:], n2e[:], ACTF.Sqrt)
    nc.vector.scalar_tensor_tensor(
        mm[:], n2[:], 1.0, qs[:], op0=ALU.add, op1=ALU.mult
    )
    nc.vector.reciprocal(rm[:], mm[:])
    nc.vector.tensor_mul(fc[:], n2[:], rm[:])
    # v = s * f (broadcast f over v)
    f_b = _ap(fc[:], [[1, C], [0, V]])
    s3 = s_in.rearrange("p (c v) -> p c v", v=V)
    nc.vector.tensor_mul(v_out[:].rearrange("p (c v) -> p c v", v=V), s3, f_b)


def _emit(nc, tc, use_bias, cc_stub=False):
    xt2_d = nc.declare_dram_parameter("xt2", [96, NT * 128], BF16, isOutput=False)
    wbd_d = nc.declare_dram_parameter("wbd", [96, NT * 640], BF16, isOutput=False)
    w2d_d = nc.declare_dram_parameter("w2d", [96, NT * CV], BF16, isOutput=False)
    if use_bias:
        bias_d = nc.declare_dram_parameter("biasr", [128, IQ * C], BF16, isOutput=False)
    out_d = nc.declare_dram_parameter("out", [128, CV], F32, isOutput=True)

    with (
        tc.tile_pool(name="const", bufs=1) as cp,
        tc.tile_pool(name="small", bufs=1) as sp,
        tc.tile_pool(name="ring", bufs=2) as rp,
        tc.tile_pool(name="gscr", bufs=1) as gp_scr,
        tc.tile_pool(name="ps0", bufs=1, space="PSUM") as ps0p,
        tc.tile_pool(name="psg", bufs=3, space="PSUM") as psgp,
        tc.tile_pool(name="dram", bufs=1, space="DRAM") as dp,
    ):
        xt2 = cp.tile([96, NT * 128], BF16, tag="xt2")
        wbd = cp.tile([96, NT * 640], BF16, tag="wbd")
        w2d = cp.tile([96, NT * CV], BF16, tag="w2d")
        uhat = cp.tile([128, IQ * CV], BF16, tag="uhat")

        # xt2 + first wbd chunk first (early gen groups), then w2d (round 0),
        # then the rest of wbd
        tl = NT // 4  # 6 tiles per load chunk
        nc.sync.dma_start(xt2[:], xt2_d[:])
        nc.sync.dma_start(wbd[:, 0 : tl * 640], wbd_d[:, 0 : tl * 640])
        nc.sync.dma_start(w2d[:], w2d_d[:])
        for j in range(1, 4):
            nc.sync.dma_start(
                wbd[:, j * tl * 640 : (j + 1) * tl * 640],
                wbd_d[:, j * tl * 640 : (j + 1) * tl * 640],
            )
        if use_bias:
            biasr = cp.tile([128, IQ * C], BF16, tag="biasr")
            nc.sync.dma_start(biasr[:], bias_d[:])

        # persistent small tiles
        warm = sp.tile([128, 1], F32, tag="warm")
        nc.vector.memset(warm[:], 1.0)

        def prewarm(func):
            # dummy op so the ACT table set loads off the critical path
            nc.scalar.activation(warm[:], warm[:], func)

        prewarm(ACTF.Sqrt)
        v_f = sp.tile([128, CV], F32, tag="v_f")
        v_b = sp.tile([128, CV], BF16, tag="v_b")
        s_part = sp.tile([128, CV], F32, tag="s_part")
        s_pd = sp.tile([128, CV], F32, tag="s_pd")
        s_pg = sp.tile([128, CV], F32, tag="s_pg")
        s_tot = sp.tile([128, CV], F32, tag="s_tot")
        ta = sp.tile([128, IQ * C], BF16, tag="ta")   # raw/logits/exp rotating
        tb = sp.tile([128, IQ * C], BF16, tag="tb")
        zsum = sp.tile([128, IQ], F32, tag="zsum")
        rz = sp.tile([128, IQ], F32, tag="rz")
        cw2 = sp.tile([128, IQ * C * 2], BF16, tag="cw2")

        def all_reduce(rnd, src, dst):
            ccin = dp.tile([128, CV], F32, tag=f"ccin{rnd}")
            ccout = dp.tile([128, CV], F32, tag=f"ccout{rnd}")
            nc.sync.dma_start(ccin[:], src[:])
            if cc_stub:
                nc.sync.dma_start(ccout[:], ccin[:])
            else:
                nc.gpsimd.collective_compute(
                    "AllReduce",
                    ALU.add,
                    replica_groups=REPLICA_GROUPS,
                    ins=[ccin.opt()],
                    outs=[ccout.opt()],
                )
            nc.sync.dma_start(dst[:], ccout[:])

        # ---- u_hat generation: block-diag matmuls, 4 i's per PSUM chunk
        def gen_group(g):
            ps = psgp.tile([128, 1024], F32, tag="psg")
            t, s = divmod(g, 3)
            for half in range(2):     # i0/i1 cols then i2/i3 cols
                nc.tensor.matmul(
                    ps[:, half * 512 :][:, :320],
                    xt2[s * 32 : (s + 1) * 32, t * 128 : (t + 1) * 128],
                    wbd[s * 32 : (s + 1) * 32, t * 640 + half * 320 :][:, :320],
                    start=True,
                    stop=True,
                )
            src = ps[:].rearrange("p (b x) -> p b x", b=2)[:, :, :320]
            dst = uhat[:, g * 640 : (g + 1) * 640].rearrange(
                "p (b x) -> p b x", b=2
            )
            if g < 16:
                nc.vector.tensor_copy(dst, src)
            else:
                nc.scalar.copy(dst, src)

        # early gen groups fill the pre-v0 idle window on PE/DVE
        for g in range(8):
            gen_group(g)

        # ---- round 0: s0 = sum_i softmax_c(bias)[i,c] * u_hat; the softmax
        # weights are folded into w2d on the host (uniform 1/C for zero bias)
        ps0 = ps0p.tile([128, CV], F32, tag="ps0")
        for t in range(NT):
            nc.tensor.matmul(
                ps0[:],
                xt2[:, t * 128 : (t + 1) * 128],
                w2d[:, t * CV : (t + 1) * CV],
                start=(t == 0),
                stop=(t == NT - 1),
            )
        nc.vector.tensor_copy(s_part[:], ps0[:])
        all_reduce(0, s_part, s_tot)
        _squash(nc, sp, s_tot[:], v_b)
        prewarm(ACTF.Exp)

        # ---- routing rounds 1, 2 (logits phase, then weighted-sum phase)
        GPL = set()   # logits chunks on GPSIMD (softmax pieces deferred)
        GPW = {0, 3, 6, 9, 12, 15}   # weighted-sum chunks on GPSIMD
        lg1 = None
        for rnd in (1, 2):
            raw = ta if rnd == 1 else tb
            et = tb if rnd == 1 else ta
            acc_state = {"d": True, "g": True}

            def ws_chunk(k):
                eng = nc.gpsimd if k in GPW else nc.vector
                uh = uhat[:, k * RC : (k + 1) * RC]
                cw2k = cw2[:, k * RCH * C * 2 : (k + 1) * RCH * C * 2]
                if k in GPW:
                    prod = gp_scr.tile([128, RC], BF16, tag="ringg")
                    tre2 = gp_scr.tile([128, 2240], BF16, tag="treeg")
                else:
                    prod = rp.tile([128, RC], BF16, tag="ring")
                    tre2 = rp.tile([128, 2240], BF16, tag="tree")
                eng.tensor_mul(
                    prod[:].rearrange("p (i c a b) -> p i c a b", c=C, a=8, b=2),
                    uh.rearrange("p (i c a b) -> p i c a b", c=C, a=8, b=2),
                    _ap(cw2k, [[20, RCH], [2, C], [0, 8], [1, 2]]),
                )
                eng.tensor_add(tre2[:, 0:1280], prod[:, 0:1280], prod[:, 1280:2560])
                eng.tensor_add(tre2[:, 1280:1920], tre2[:, 0:640], tre2[:, 640:1280])
                eng.tensor_add(
                    tre2[:, 1920:2240], tre2[:, 1280:1600], tre2[:, 1600:1920]
                )
                if k in GPW:
                    if acc_state["g"]:
                        nc.gpsimd.tensor_add(
                            s_pg[:], tre2[:, 1920:2080], tre2[:, 2080:2240]
                        )
                        acc_state["g"] = False
                    else:
                        nc.gpsimd.tensor_add(s_pg[:], s_pg[:], tre2[:, 1920:2080])
                        nc.gpsimd.tensor_add(s_pg[:], s_pg[:], tre2[:, 2080:2240])
                else:
                    if acc_state["d"]:
                        nc.vector.tensor_add(
                            s_pd[:], tre2[:, 1920:2080], tre2[:, 2080:2240]
                        )
                        acc_state["d"] = False
                    else:
                        nc.vector.tensor_add(s_pd[:], s_pd[:], tre2[:, 1920:2080])
                        nc.vector.tensor_add(s_pd[:], s_pd[:], tre2[:, 2080:2240])

            def softmax_tail(k, n=1):
                ks = slice(k * RCH * C, (k + n) * RCH * C)
                kz = slice(k * RCH, (k + n) * RCH)
                nc.vector.tensor_reduce(
                    zsum[:, kz],
                    et[:, ks].rearrange("p (i c) -> p i c", c=C),
                    axis=AX.X,
                    op=ALU.add,
                )
                nc.vector.reciprocal(rz[:, kz], zsum[:, kz])
                cw2k = cw2[:, k * RCH * C * 2 : (k + n) * RCH * C * 2]
                nc.gpsimd.tensor_mul(
                    cw2k.rearrange("p (i c t) -> p i c t", c=C, t=2),
                    _ap(et[:, ks.start :], [[10, n * RCH], [1, C], [0, 2]]),
                    _ap(rz[:, k * RCH :], [[1, n * RCH], [0, C], [0, 2]]),
                )

            # phase 1: logits chunks + softmax pieces (pipelined across engines)
            for k in range(NRC):
                if rnd == 1 and k >= 2:
                    # u_hat generation interleaved (groups 0-7 emitted pre-s0)
                    for g in range(4 * k, 4 * k + 4):
                        gen_group(g)
                eng = nc.gpsimd if k in GPL else nc.vector
                ks = slice(k * RCH * C, (k + 1) * RCH * C)
                uh = uhat[:, k * RC : (k + 1) * RC]
                if k in GPL:
                    tmp = gp_scr.tile([128, RC], BF16, tag="ringg")
                    tre = gp_scr.tile([128, 2240], BF16, tag="treeg")
                else:
                    tmp = rp.tile([128, RC], BF16, tag="ring")
                    tre = rp.tile([128, 2240], BF16, tag="tree")
                vb3 = _ap(v_b[:], [[0, RCH], [16, C], [1, V]])
                eng.tensor_mul(
                    tmp[:].rearrange("p (i c v) -> p i c v", c=C, v=V),
                    uh.rearrange("p (i c v) -> p i c v", c=C, v=V),
                    vb3,
                )
                t16 = tmp[:].rearrange("p (x v) -> p x v", v=16)
                t8 = tre[:, 0:1280].rearrange("p (x v) -> p x v", v=8)
                t4 = tre[:, 1280:1920].rearrange("p (x v) -> p x v", v=4)
                t2 = tre[:, 1920:2240].rearrange("p (x v) -> p x v", v=2)
                eng.tensor_add(t8, t16[:, :, 0:8], t16[:, :, 8:16])
                eng.tensor_add(t4, t8[:, :, 0:4], t8[:, :, 4:8])
                eng.tensor_add(t2, t4[:, :, 0:2], t4[:, :, 2:4])
                eng.tensor_add(
                    raw[:, ks],
                    t2[:, :, 0:1].rearrange("p x v -> p (x v)"),
                    t2[:, :, 1:2].rearrange("p x v -> p (x v)"),
                )
                if rnd == 1 and use_bias:
                    eng.tensor_add(raw[:, ks], raw[:, ks], biasr[:, ks])
                if rnd == 2:
                    eng.tensor_add(raw[:, ks], raw[:, ks], lg1[:, ks])
                nc.scalar.activation(et[:, ks], raw[:, ks], ACTF.Exp)
                if k in GPW:
                    softmax_tail(k)
                    ws_chunk(k)
                elif k % 3 == 2:
                    softmax_tail(k - 1, n=2)
            # deferred softmax pieces of GPSIMD logits chunks (so the DVE
            # queue never stalls mid-phase waiting on a slow GPSIMD chunk)
            for k in sorted(GPL):
                softmax_tail(k)
            # phase 2: weighted-sum chunks, per-engine partial accumulators
            prewarm(ACTF.Sqrt)
            for k in range(NRC):
                if k not in GPW:
                    ws_chunk(k)
            if rnd == 1:
                lg1 = raw
            nc.vector.tensor_add(s_part[:], s_pd[:], s_pg[:])
            all_reduce(rnd, s_part, s_tot)
            _squash(nc, sp, s_tot[:], v_b if rnd == 1 else v_f)
            if rnd == 1:
                prewarm(ACTF.Exp)

        nc.sync.dma_start(out_d[:], v_f[:])


_PROGRAMS = {}


def _get_program(use_bias=False, cc_stub=False):
    key = (use_bias, cc_stub)
    if key not in _PROGRAMS:
        nc = bacc.Bacc(
            "TRN2", target_bir_lowering=False, debug=False, num_devices=8
        )
        with tile.TileContext(nc) as tc:
            _emit(nc, tc, use_bias, cc_stub)
        nc.compile()
        _PROGRAMS[key] = nc
    return _PROGRAMS[key]


def make_in_maps(inputs, W, bias):
    assert tuple(np.shape(inputs)) == (B, I, D), np.shape(inputs)
    assert tuple(np.shape(W)) == (I, C, D, V), np.shape(W)
    assert tuple(np.shape(bias)) == (1, I, C, 1), np.shape(bias)
    use_bias = bool(np.any(np.asarray(bias)))
    in_maps = []
    for k in range(8):
        bh, iq = k // 4, k % 4
        xs = np.asarray(inputs[bh * NB : (bh + 1) * NB, iq * IQ : (iq + 1) * IQ, :])
        ws = np.asarray(W[iq * IQ : (iq + 1) * IQ])  # [288, 10, 8, 16]

        xT = xs.reshape(NB, IQ * D).T  # [2304, 128] rows (i,d)
        xt2 = xT.reshape(NT, 96, NB).transpose(1, 0, 2).reshape(96, NT * NB)

        Wt = ws.transpose(0, 2, 1, 3)  # [288, 8, 10, 16] (i, d, c, v)
        bs = np.asarray(bias[0, iq * IQ : (iq + 1) * IQ, :, 0], dtype=np.float64)
        eb = np.exp(bs - bs.max(axis=1, keepdims=True))
        cb = (eb / eb.sum(axis=1, keepdims=True)).astype(np.float32)  # [288, 10]
        Wt_s = Wt * cb[:, None, :, None]  # fold round-0 softmax into s0 weights
        w2dense = Wt_s.reshape(IQ * D, CV)  # [(i,d), (c,v)]
        w2d = w2dense.reshape(NT, 96, CV).transpose(1, 0, 2).reshape(96, NT * CV)

        bd = np.zeros((NG, 32, 640), dtype=np.float32)
        Wg = Wt.reshape(NG, 4, D, CV)
        for j in range(4):
            bd[:, j * D : (j + 1) * D, j * CV : (j + 1) * CV] = Wg[:, j]
        wbd = bd.reshape(NT, 96, 640).transpose(1, 0, 2).reshape(96, NT * 640)

        m = {
            "xt2": np.ascontiguousarray(xt2).astype(ml_dtypes.bfloat16),
            "wbd": np.ascontiguousarray(wbd).astype(ml_dtypes.bfloat16),
            "w2d": np.ascontiguousarray(w2d).astype(ml_dtypes.bfloat16),
        }
        if use_bias:
            bs = np.asarray(bias[0, iq * IQ : (iq + 1) * IQ, :, 0])
            biasr = np.broadcast_to(bs.reshape(1, IQ * C), (128, IQ * C))
            m["biasr"] = np.ascontiguousarray(biasr).astype(ml_dtypes.bfloat16)
        in_maps.append(m)
    return use_bias, in_maps


def run(inputs, W, bias, **kw):
    use_bias, in_maps = make_in_maps(inputs, W, bias)
    nc = _get_program(use_bias)
    res = run_bass_kernel_spmd(nc, in_maps, core_ids=list(range(8)), **kw)
    outs = res.results
    o0 = np.asarray(outs[0]["out"], dtype=np.float32).reshape(NB, C, V)
    o1 = np.asarray(outs[4]["out"], dtype=np.float32).reshape(NB, C, V)
    return np.concatenate([o0, o1], axis=0), res


def kernel(inputs, W, bias):
    out, _ = run(inputs, W, bias)
    return out

